# revision 3
# baseline (speedup 1.0000x reference)
"""GNN message-passing (e3nn-style convolution) on 8 Trainium2 cores.

Sharding: nodes partitioned across 8 cores (3750/core); each edge lives on the
core owning edge_dst, sorted into 128-node dst windows so the scatter-reduce is
a sequence of one-hot matmuls accumulating in PSUM. lin1'd node features are
computed on every core (replicated), written to HBM in bf16, gathered per-edge
with dma_gather. Per-edge FC weights via PE matmuls; the tensor product is
elementwise bf16 on DVE; lin2/sc paths run as f32r matmuls.
"""
import math
import sys

sys.path.insert(0, "/opt/trn_rl_repo")

import numpy as np
import ml_dtypes

import concourse.bacc as bacc
import concourse.mybir as mybir
import concourse.tile as tile
from concourse import library_config
from concourse.bass_utils import run_bass_kernel_spmd

N = 30000
MUL = 64
NES = 8
FCN = 64
N_CORES = 8
NPC = 3750
WIN = 128
N_WIN = 30
NPC_PAD = N_WIN * WIN          # 3840
CHUNK = 128
GC = 8                         # chunks per group
GROUP = CHUNK * GC             # 1024 edges
NT = 235                       # node tiles of 128
NPAD = NT * 128                # 30080

INV_SQRT3 = 1.0 / math.sqrt(3.0)
SILU_NORM = 1.679
C_S = math.sin(math.pi / 8.0)
C_X = math.cos(math.pi / 8.0)
INV_MUL = 1.0 / math.sqrt(MUL)
INV2 = 1.0 / math.sqrt(2 * MUL)
INV_NEI = 1.0 / math.sqrt(20.0)

F32 = mybir.dt.float32
F32R = mybir.dt.float32r
BF16 = mybir.dt.bfloat16
I16 = mybir.dt.int16
MULT = mybir.AluOpType.mult
ISEQ = mybir.AluOpType.is_equal
SILU = mybir.ActivationFunctionType.Silu
BF = ml_dtypes.bfloat16


def _interleave(w, scale):
    out = np.zeros((3 * MUL, 3 * MUL), np.float32)
    for c in range(3):
        out[c::3, c::3] = w * scale
    return out


def _host_weights(inp):
    w = {}
    Wy = np.zeros((256, 256), np.float32)
    Wy[:64, :64] = np.asarray(inp["lin1_w0"]) * INV_MUL
    Wy[64:, 64:] = _interleave(np.asarray(inp["lin1_w1"]), INV_MUL)
    w["W_y"] = Wy.astype(BF)
    Ws = np.zeros((256, 256), np.float32)
    Ws[:64, :64] = np.asarray(inp["sc_w0"]) * (INV_MUL * C_S)
    Ws[64:, 64:] = _interleave(np.asarray(inp["sc_w1"]), INV_MUL * C_S)
    w["W_sc"] = Ws
    w["fc1"] = (np.asarray(inp["fc_w1"]) * (1.0 / math.sqrt(NES))).astype(np.float32)
    s2 = SILU_NORM / math.sqrt(FCN)
    fc2 = np.asarray(inp["fc_w2"], np.float32)
    W2 = np.zeros((64, 640), np.float32)
    W2[:, 0:64] = fc2[:, 0:64] * s2
    W2[:, 64:256] = np.repeat(fc2[:, 64:128] * (s2 * INV_SQRT3), 3, axis=1)
    W2[:, 256:448] = np.repeat(fc2[:, 128:192] * s2, 3, axis=1)
    W2[:, 448:640] = np.repeat(fc2[:, 192:256] * s2, 3, axis=1)
    w["fc2e"] = W2.astype(BF)
    k = INV_NEI * INV2 * C_X
    WL = np.zeros((640, 256), np.float32)
    WL[0:64, 0:64] = np.asarray(inp["lin2_w00"]) * k
    w01 = np.asarray(inp["lin2_w01"]) * k
    WL[64:256, 0:64] = np.repeat(w01, 3, axis=0)
    WL[256:448, 64:256] = _interleave(np.asarray(inp["lin2_w10"]), k)
    WL[448:640, 64:256] = _interleave(np.asarray(inp["lin2_w11"]), k)
    w["W_l2"] = WL
    return w


def _host_edges(inp):
    src = np.asarray(inp["edge_src"]).astype(np.int64)
    dst = np.asarray(inp["edge_dst"]).astype(np.int64)
    ea = np.asarray(inp["edge_attr"], np.float32)
    es = np.asarray(inp["edge_scalars"], np.float32)

    core = dst // NPC
    per_core = []
    kmax = np.zeros(N_WIN, np.int64)
    for c in range(N_CORES):
        m = core == c
        d_loc = dst[m] - c * NPC
        order = np.argsort(d_loc, kind="stable")
        idx_e = np.nonzero(m)[0][order]
        d_loc = d_loc[order]
        win = d_loc // WIN
        cnt = np.bincount(win, minlength=N_WIN)
        kmax = np.maximum(kmax, (cnt + CHUNK - 1) // CHUNK)
        per_core.append((idx_e, d_loc, cnt))
    kmax = np.maximum(kmax, 1)
    k_tot = int(kmax.sum())
    k_pad = ((k_tot + GC - 1) // GC) * GC
    e_pad = k_pad * CHUNK
    sched = []
    for wdx in range(N_WIN):
        sched += [wdx] * int(kmax[wdx])
    sched += [N_WIN - 1] * (k_pad - k_tot)

    cores = []
    for c in range(N_CORES):
        idx_e, d_loc, cnt = per_core[c]
        src16 = np.zeros(e_pad, np.int16)
        drel = np.full(e_pad, -1.0, np.float32)
        ea_s = np.zeros((e_pad, 4), np.float32)
        es_s = np.zeros((e_pad, NES), np.float32)
        pos = 0
        start = 0
        for wdx in range(N_WIN):
            n = int(cnt[wdx])
            sl = slice(start, start + n)
            src16[pos:pos + n] = src[idx_e[sl]].astype(np.int16)
            drel[pos:pos + n] = (d_loc[sl] - wdx * WIN).astype(np.float32)
            ea_s[pos:pos + n] = ea[idx_e[sl]]
            es_s[pos:pos + n] = es[idx_e[sl]]
            start += n
            pos += int(kmax[wdx]) * CHUNK
        cores.append(dict(
            idx=np.ascontiguousarray(np.tile(src16.reshape(-1, 16).T, (8, 1))),
            esT=np.ascontiguousarray(es_s.T),
            ea=np.ascontiguousarray(
                ea_s.reshape(k_pad, CHUNK, 4).transpose(1, 0, 2).astype(BF)),
            drel=np.ascontiguousarray(drel.reshape(k_pad, CHUNK).T.astype(BF)),
        ))
    return cores, sched, k_pad, e_pad


def _build_program(sched, k_pad, e_pad):
    n_groups = k_pad // GC
    nc = bacc.Bacc("TRN2", target_bir_lowering=False, debug=False,
                   num_devices=N_CORES)
    xT_bf = nc.dram_tensor("xT_bf", [256, NPAD], BF16, kind="ExternalInput")
    xT_own = nc.dram_tensor("xT_own", [256, NPC_PAD], F32, kind="ExternalInput")
    W_y = nc.dram_tensor("W_y", [256, 256], BF16, kind="ExternalInput")
    W_sc = nc.dram_tensor("W_sc", [256, 256], F32, kind="ExternalInput")
    W_l2 = nc.dram_tensor("W_l2", [640, 256], F32, kind="ExternalInput")
    fc1_d = nc.dram_tensor("fc1", [8, 64], F32, kind="ExternalInput")
    fc2_d = nc.dram_tensor("fc2e", [64, 640], BF16, kind="ExternalInput")
    ident = nc.dram_tensor("ident", [128, 128], F32, kind="ExternalInput")
    iota_d = nc.dram_tensor("iota", [128, GC * 128], BF16, kind="ExternalInput")
    idx_d = nc.dram_tensor("idx", [128, e_pad // 16], I16, kind="ExternalInput")
    esT_d = nc.dram_tensor("esT", [8, e_pad], F32, kind="ExternalInput")
    ea_d = nc.dram_tensor("ea", [128, k_pad, 4], BF16, kind="ExternalInput")
    drel_d = nc.dram_tensor("drel", [128, k_pad], BF16, kind="ExternalInput")
    y_hbm = nc.dram_tensor("y", [NPAD, 256], BF16)
    out_d = nc.dram_tensor("out", [NPC_PAD, 256], F32, kind="ExternalOutput")

    with tile.TileContext(nc) as tc:
        nc.gpsimd.load_library(library_config.mlp)
        with tc.tile_pool(name="const", bufs=1) as cp:
            w_y = cp.tile([128, 2, 256], BF16, tag="wy")
            w_sc = cp.tile([128, 2, 256], F32, tag="wsc")
            w_l2 = cp.tile([128, 5, 256], F32, tag="wl2")
            for h in range(2):
                nc.sync.dma_start(w_y[:, h, :], W_y[h * 128:(h + 1) * 128, :])
                nc.sync.dma_start(w_sc[:, h, :], W_sc[h * 128:(h + 1) * 128, :])
            for b in range(5):
                nc.sync.dma_start(w_l2[:, b, :], W_l2[b * 128:(b + 1) * 128, :])
            w_fc1 = cp.tile([8, 64], F32, tag="fc1")
            nc.sync.dma_start(w_fc1[:], fc1_d[:])
            w_fc2 = cp.tile([64, 640], BF16, tag="fc2")
            nc.sync.dma_start(w_fc2[:], fc2_d[:])
            idn = cp.tile([128, 128], F32, tag="idn")
            nc.sync.dma_start(idn[:], ident[:])
            iot = cp.tile([128, GC, 128], BF16, tag="iota")
            nc.sync.dma_start(iot[:], iota_d[:].rearrange("p (k j) -> p k j", j=128))
            idx_sb = cp.tile([128, e_pad // 16], I16, tag="idx")
            nc.sync.dma_start(idx_sb[:], idx_d[:])
            ea_sb = cp.tile([128, k_pad, 4], BF16, tag="ea")
            nc.sync.dma_start(ea_sb[:], ea_d[:])
            dr_sb = cp.tile([128, k_pad], BF16, tag="drel")
            nc.sync.dma_start(dr_sb[:], drel_d[:])

            # ---- node phase: y = lin1(x) in bf16 -> HBM ----
            with tc.tile_pool(name="nsb", bufs=3) as npool, \
                 tc.tile_pool(name="nps", bufs=2, space="PSUM") as npp:
                for t in range(NT):
                    sl = slice(t * 128, (t + 1) * 128)
                    xt = npool.tile([128, 2, 128], BF16, tag="xt")
                    for h in range(2):
                        nc.sync.dma_start(xt[:, h, :], xT_bf[h * 128:(h + 1) * 128, sl])
                    yp = npp.tile([128, 256], F32, tag="yp")
                    nc.tensor.matmul(yp[:], xt[:, 0, :], w_y[:, 0, :],
                                     start=True, stop=False)
                    nc.tensor.matmul(yp[:], xt[:, 1, :], w_y[:, 1, :],
                                     start=False, stop=True)
                    ysb = npool.tile([128, 256], BF16, tag="ysb")
                    nc.scalar.copy(ysb[:], yp[:])
                    nc.sync.dma_start(y_hbm[sl, :], ysb[:])

            # ---- edge phase ----
            with tc.tile_pool(name="esb", bufs=2) as ep, \
                 tc.tile_pool(name="zsb", bufs=2) as zp_sb, \
                 tc.tile_pool(name="hps", bufs=2, space="PSUM") as hpp, \
                 tc.tile_pool(name="wps", bufs=1, space="PSUM") as wpp, \
                 tc.tile_pool(name="zps", bufs=1, space="PSUM") as zpp, \
                 tc.tile_pool(name="ops", bufs=1, space="PSUM") as opp:
                cur_win = -1
                za = zb = None

                def close_window(wdx):
                    z_sb = zp_sb.tile([128, 640], F32, tag="z")
                    nc.scalar.copy(z_sb[:, 0:512], za[:])
                    nc.scalar.copy(z_sb[:, 512:640], zb[:])
                    zT = zp_sb.tile([128, 5, 128], F32, tag="zT")
                    for b in range(5):
                        tp = opp.tile([128, 128], F32, tag="tp")
                        nc.tensor.transpose(tp[:], z_sb[:, b * 128:(b + 1) * 128], idn[:])
                        nc.scalar.copy(zT[:, b, :], tp[:])
                    xsc = ep.tile([128, 2, 128], F32, tag="xsc")
                    wsl = slice(wdx * 128, (wdx + 1) * 128)
                    for h in range(2):
                        nc.sync.dma_start(xsc[:, h, :], xT_own[h * 128:(h + 1) * 128, wsl])
                    o = opp.tile([128, 256], F32, tag="o")
                    for b in range(5):
                        nc.tensor.matmul(o[:], zT[:, b, :], w_l2[:, b, :],
                                         start=(b == 0), stop=False)
                    for h in range(2):
                        nc.tensor.matmul(o[:], xsc[:, h, :], w_sc[:, h, :],
                                         start=False, stop=(h == 1))
                    osb = zp_sb.tile([128, 256], F32, tag="osb")
                    nc.scalar.copy(osb[:], o[:])
                    nc.sync.dma_start(out_d[wsl, :], osb[:])

                for g in range(n_groups):
                    est = ep.tile([8, GROUP], F32, tag="est")
                    nc.sync.dma_start(est[:], esT_d[:, g * GROUP:(g + 1) * GROUP])
                    xg = ep.tile([128, GC, 256], BF16, tag="xg")
                    nc.gpsimd.dma_gather(xg[:], y_hbm[:], idx_sb[:, g * 64:(g + 1) * 64],
                                         GROUP, GROUP, 256)
                    hT = ep.tile([64, GROUP], BF16, tag="hT")
                    for h in range(2):
                        hp = hpp.tile([64, 512], F32, tag="hp")
                        nc.tensor.matmul(hp[:], w_fc1[:],
                                         est[:, h * 512:(h + 1) * 512],
                                         start=True, stop=True)
                        nc.scalar.activation(hT[:, h * 512:(h + 1) * 512], hp[:], SILU)
                    ksl = slice(g * GC, (g + 1) * GC)
                    oh = ep.tile([128, GC, 128], BF16, tag="oh")
                    nc.vector.tensor_tensor(
                        oh[:], iot[:],
                        dr_sb[:, ksl].unsqueeze(2).broadcast_to([128, GC, 128]), ISEQ)
                    G = ep.tile([128, GC, 640], BF16, tag="G")
                    ea0 = ea_sb[:, ksl, 0:1]
                    ea1 = ea_sb[:, ksl, 1:4]
                    nc.vector.tensor_tensor(
                        G[:, :, 0:64], xg[:, :, 0:64],
                        ea0.broadcast_to([128, GC, 64]), MULT)
                    nc.vector.tensor_tensor(
                        G[:, :, 64:256].rearrange("p k (u c) -> p k u c", c=3),
                        xg[:, :, 64:256].rearrange("p k (u c) -> p k u c", c=3),
                        ea1.unsqueeze(2).broadcast_to([128, GC, 64, 3]), MULT)
                    nc.vector.tensor_tensor(
                        G[:, :, 256:448].rearrange("p k (u c) -> p k u c", c=3),
                        xg[:, :, 0:64].unsqueeze(3).broadcast_to([128, GC, 64, 3]),
                        ea1.unsqueeze(2).broadcast_to([128, GC, 64, 3]), MULT)
                    nc.vector.tensor_tensor(
                        G[:, :, 448:640], xg[:, :, 64:256],
                        ea0.broadcast_to([128, GC, 192]), MULT)
                    mids = ep.tile([128, GC, 640], BF16, tag="mids")
                    for k in range(GC):
                        kk = g * GC + k
                        wdx = sched[kk]
                        wp = wpp.tile([128, 640], F32, tag="wp")
                        nc.tensor.matmul(wp[:, 0:512], hT[:, k * 128:(k + 1) * 128],
                                         w_fc2[:, 0:512], start=True, stop=True)
                        nc.tensor.matmul(wp[:, 512:640], hT[:, k * 128:(k + 1) * 128],
                                         w_fc2[:, 512:640], start=True, stop=True)
                        nc.vector.tensor_tensor(mids[:, k, :], wp[:], G[:, k, :], MULT)
                        if wdx != cur_win:
                            if cur_win >= 0:
                                close_window(cur_win)
                            cur_win = wdx
                            za = zpp.tile([128, 512], F32, tag="za")
                            zb = zpp.tile([128, 128], F32, tag="zb")
                            first = True
                        else:
                            first = False
                        last = (kk + 1 == k_pad) or (sched[kk + 1] != wdx)
                        nc.tensor.matmul(za[:], oh[:, k, :], mids[:, k, 0:512],
                                         start=first, stop=last)
                        nc.tensor.matmul(zb[:], oh[:, k, :], mids[:, k, 512:640],
                                         start=first, stop=last)
                close_window(cur_win)
    nc.compile()
    return nc


def _prep(inputs):
    wts = _host_weights(inputs)
    cores, sched, k_pad, e_pad = _host_edges(inputs)
    x = np.asarray(inputs["node_input"], np.float32)
    xT = np.zeros((256, NPAD), np.float32)
    xT[:, :N] = x.T
    shared = {
        "xT_bf": xT.astype(BF),
        "W_y": wts["W_y"], "W_sc": wts["W_sc"], "W_l2": wts["W_l2"],
        "fc1": wts["fc1"], "fc2e": wts["fc2e"],
        "ident": np.eye(128, dtype=np.float32),
        "iota": np.tile(np.arange(128, dtype=np.float32).astype(BF), (128, GC)),
    }
    in_maps = []
    for c in range(N_CORES):
        m = dict(shared)
        xo = np.zeros((256, NPC_PAD), np.float32)
        xo[:, :NPC] = x[c * NPC:(c + 1) * NPC].T
        m["xT_own"] = xo
        m["idx"] = cores[c]["idx"]
        m["esT"] = cores[c]["esT"]
        m["ea"] = cores[c]["ea"]
        m["drel"] = cores[c]["drel"]
        in_maps.append(m)
    return in_maps, sched, k_pad, e_pad


def _run(inputs, trace=False):
    in_maps, sched, k_pad, e_pad = _prep(inputs)
    nc = _build_program(sched, k_pad, e_pad)
    res = run_bass_kernel_spmd(nc, in_maps, core_ids=list(range(N_CORES)),
                               trace=trace)
    out = np.zeros((N, 256), np.float32)
    for c in range(N_CORES):
        out[c * NPC:(c + 1) * NPC] = res.results[c]["out"][:NPC]
    return out, res


def kernel(**inputs):
    out, _ = _run(inputs, trace=False)
    return out
